# revision 59
# baseline (speedup 1.0000x reference)
"""Causal multi-head attention block (B=4, S=2048, D=768, H=12, Dh=64)
distributed over 8 NeuronCores: core = (batch, head-group), each core
computes its 6 heads end-to-end plus its partial output projection;
host sums the two partials per batch and adds the bias.

Self-contained: hardcodes all shapes; no sibling imports.
"""

import numpy as np

B, S, D = 4, 2048, 768
H, DH = 12, 64
G = 384          # channels per head group (6 heads)
NPAIR = 3        # head pairs per core
NSC = 4          # 512-wide query windows
W = 512
NST = 16         # 128-row s-tiles
NDC = 6          # 128-row D chunks

# packed-weights layout (per-partition column offsets in the wts tile)
MK0 = 0
WV0 = 128                    # full wv (6 chunks x 384)
WO0 = 2432                   # wo (3 chunks x 768)
WX = 4736
SC8 = 32.0                   # fp8 q/k weight pre-scale (avoids e4m3
                             # subnormals at w~0.02); undone in exp scale

_PROGRAM = None
PROFILE = False
PROFILE_DIR = None
LAST_RESULT = None


def _split_waits(nc, max_waits=1, max_updates=1):
    """This container's walrus rejects instructions carrying more than one
    semaphore wait/update ("Too many sync wait commands").  Move excess
    waits onto NoOps inserted before the owning instruction (same engine)
    and excess updates onto NoOps inserted after."""
    import concourse.mybir as mybir

    counter = [0]

    def nop(engine, waits, updates):
        counter[0] += 1
        n = mybir.InstNoOp(name=f"wsplit_nop_{counter[0]}", ins=[], outs=[])
        n.engine = engine
        n.sync_info = mybir.SyncInfo(on_wait=waits, on_update=updates)
        return n

    for bb in nc.main_func.blocks:
        out = []
        changed = False
        for ins in bb.instructions:
            si = ins.sync_info
            waits = list(si.on_wait) if si and si.on_wait else []
            updates = list(si.on_update) if si and si.on_update else []
            pre, post = [], []
            if len(waits) > max_waits:
                keep = waits[:max_waits - 1] if max_waits > 1 else []
                rest = waits[len(keep):]
                while rest:
                    chunk, rest = rest[:max_waits], rest[max_waits:]
                    pre.append(chunk)
                waits = keep
                changed = True
            if len(updates) > max_updates:
                rest = updates[max_updates:]
                updates = updates[:max_updates]
                while rest:
                    chunk, rest = rest[:max_updates], rest[max_updates:]
                    post.append(chunk)
                changed = True
            if pre or post:
                ins.sync_info = mybir.SyncInfo(
                    on_wait=waits, on_update=updates)
            for w in pre:
                out.append(nop(ins.engine, w, []))
            out.append(ins)
            for u in post:
                out.append(nop(ins.engine, [], u))
        if changed:
            bb.instructions = out


def _install_profile_hooks():
    """Dev-only (PROFILE=True): register the NTFF profile hook that the
    agent image's antenv lacks, and stub out the artifact upload."""
    import sys
    import types

    try:
        from antenv.axon_hooks import get_axon_ntff_profile_hook  # noqa: F401
    except ImportError:
        import antenv
        from trn_agent_boot import trn_boot

        hook = trn_boot._ntff_profile_via_ctypes("/opt/axon/libaxon_pjrt.so")
        mod = types.ModuleType("antenv.axon_hooks")
        mod._hook = hook
        mod.get_axon_ntff_profile_hook = lambda: mod._hook
        mod.set_axon_ntff_profile_hook = lambda h: setattr(mod, "_hook", h)
        sys.modules["antenv.axon_hooks"] = mod
        antenv.axon_hooks = mod

    from concourse import bass_utils

    bass_utils.upload_artifacts = lambda tmpdir: "local://" + tmpdir


def _build_program():
    import concourse.bass as bass
    import concourse.mybir as mybir
    import concourse.tile as tile

    f16 = mybir.dt.float16
    f32 = mybir.dt.float32
    f8 = mybir.dt.float8e4

    nc = bass.Bass()
    # xt is block-major: [128, window, chunk*512] so each window's slice is
    # one contiguous 6KB-per-partition DMA.  xt8/wqk8 are fp8 copies used
    # only by the DoubleRow q/k projections.
    xt_d = nc.declare_dram_parameter("xt", [128, NSC, NDC, W], f16,
                                     isOutput=False)
    xt8_d = nc.declare_dram_parameter("xt8", [128, NSC, NDC, W], f8,
                                      isOutput=False)
    wqk8_d = nc.declare_dram_parameter("wqk8", [128, 2 * NPAIR, NDC, 128],
                                       f8, isOutput=False)
    wts_d = nc.declare_dram_parameter("wts", [128, WX], f16, isOutput=False)
    y_d = nc.declare_dram_parameter("y", [S, D], f16, isOutput=True)

    with tile.TileContext(nc) as tc:
        with (
            tc.tile_pool(name="const", bufs=1) as const,
            tc.tile_pool(name="work", bufs=3) as work,
            tc.tile_pool(name="outp", bufs=3) as outp,
            tc.tile_pool(name="ps", bufs=2, space="PSUM") as ps,
        ):
            # ---- persistent SBUF tiles ----
            wts = const.tile([128, WX], f16, name="wts", tag="wts")
            xt = const.tile([128, NSC, NDC, W], f16, name="xt", tag="xt")
            xt8 = const.tile([128, NSC, NDC, W], f8, name="xt8", tag="xt8")
            wqk8 = const.tile([128, 2 * NPAIR, NDC, 128], f8, name="wqk8",
                              tag="wqk8")

            def xv(sc, dc, c0, c1):
                # chunk dc, window-relative cols c0..c1
                return xt[:, sc, dc, c0:c1]
            qt = [const.tile([128, S], f16, name=f"qt{p}", tag=f"qt{p}")
                  for p in range(NPAIR)]
            kt = [const.tile([128, S], f16, name=f"kt{p}", tag=f"kt{p}")
                  for p in range(NPAIR)]
            gt = [const.tile([128, S], f16, name=f"gt{p}", tag=f"gt{p}")
                  for p in range(NPAIR)]
            # vt[st]: per head h the 128 lhsT columns [v_h (64) | ones (64)]
            # so one matmul per head accumulates attn@V on out partitions
            # 0:64 and the softmax denominator (replicated) on 64:128.
            vt = [const.tile([128, 2 * NPAIR, 128], f16, name=f"vt{t}",
                             tag=f"vt{t}") for t in range(NST)]

            mkv = wts[:, MK0:MK0 + 128]

            def wvv(dc):
                return wts[:, WV0 + 384 * dc:WV0 + 384 * (dc + 1)]

            def wov(cc, half):
                b = WO0 + 768 * cc + 384 * half
                return wts[:, b:b + 384]

            # ---- input DMAs, need-ordered; both queues share one HBM
            # stream so the first-window deps (mk+pair0 qk, xt cols 0:512,
            # wv) go first and the rest rides behind compute ----
            # single queue, exact need order: the HBM stream is shared, so
            # interleaving a second queue only delays the critical set.
            # y-output DMAs ride the gpsimd queue instead.
            nc.sync.dma_start(out=wts[:, 0:WV0], in_=wts_d[:, 0:WV0])  # mk
            nc.sync.dma_start(out=wqk8[:, 0:2, :, :],
                              in_=wqk8_d[:, 0:2, :, :])
            nc.sync.dma_start(out=xt8[:, 0, :, :], in_=xt8_d[:, 0, :, :])
            nc.sync.dma_start(out=wts[:, WV0:WO0], in_=wts_d[:, WV0:WO0])
            nc.sync.dma_start(out=xt[:, 0, :, 0:128],
                              in_=xt_d[:, 0, :, 0:128])
            nc.sync.dma_start(out=xt[:, 0, :, 128:W],
                              in_=xt_d[:, 0, :, 128:W])
            nc.sync.dma_start(out=wqk8[:, 2:6, :, :],
                              in_=wqk8_d[:, 2:6, :, :])
            nc.sync.dma_start(out=xt8[:, 1, :, :], in_=xt8_d[:, 1, :, :])
            nc.sync.dma_start(out=xt[:, 1, :, :], in_=xt_d[:, 1, :, :])
            nc.sync.dma_start(out=xt8[:, 2, :, :], in_=xt8_d[:, 2, :, :])
            nc.sync.dma_start(out=xt[:, 2, :, :], in_=xt_d[:, 2, :, :])
            nc.sync.dma_start(out=xt8[:, 3, :, :], in_=xt8_d[:, 3, :, :])
            nc.sync.dma_start(out=xt[:, 3, :, :], in_=xt_d[:, 3, :, :])
            nc.sync.dma_start(out=wts[:, WO0:WX], in_=wts_d[:, WO0:WX])

            for st in range(NST):
                nc.vector.memset(vt[st][:, :, 64:128], 1.0)

            def proj_qk_unit(pair, sc):
                # fp8 DoubleRow: 3 matmuls of 2 packed 128-chunks each
                DR = mybir.MatmulPerfMode.DoubleRow
                qp = ps.tile([128, W], f32, name=f"qp{pair}_{sc}",
                             tag="sc", bufs=2)
                for g2 in range(3):
                    nc.tensor.matmul(
                        qp, wqk8[:, 2 * pair, 2 * g2:2 * g2 + 2, :],
                        xt8[:, sc, 2 * g2:2 * g2 + 2, :],
                        start=(g2 == 0), stop=(g2 == 2), perf_mode=DR)
                nc.vector.tensor_copy(
                    out=qt[pair][:, W * sc:W * (sc + 1)], in_=qp)
                kp = ps.tile([128, W], f32, name=f"kp{pair}_{sc}",
                             tag="sc", bufs=2)
                for g2 in range(3):
                    nc.tensor.matmul(
                        kp, wqk8[:, 2 * pair + 1, 2 * g2:2 * g2 + 2, :],
                        xt8[:, sc, 2 * g2:2 * g2 + 2, :],
                        start=(g2 == 0), stop=(g2 == 2), perf_mode=DR)
                nc.vector.tensor_copy(
                    out=kt[pair][:, W * sc:W * (sc + 1)], in_=kp)

            def proj_v(st):
                vp = ps.tile([128, 2 * NPAIR, 64], f32, name=f"vp{st}",
                             tag="sc", bufs=2)
                # phase keeper: "sc"-ring users otherwise come in pairs
                # (qp+kp, o0+o1, per-block sct); a lone vp alloc would make
                # two consecutive sct allocations share a slot, collapsing
                # the scores/exp double-buffer to depth 1 for that block.
                ps.tile([128, 8], f32, name=f"vph{st}", tag="sc", bufs=2)
                for dc in range(NDC):
                    nc.tensor.matmul(
                        vp,
                        xv(st // 4, dc, 128 * (st % 4), 128 * (st % 4 + 1)),
                        wvv(dc),
                        start=(dc == 0), stop=(dc == NDC - 1))
                nc.vector.tensor_copy(out=vt[st][:, :, 0:64], in_=vp)

            def outproj(st):
                o0 = ps.tile([128, G], f32, name=f"o0_{st}", tag="sc",
                             bufs=2)
                for cc in range(3):
                    nc.tensor.matmul(
                        o0, gt[cc][:, 128 * st:128 * (st + 1)], wov(cc, 0),
                        start=(cc == 0), stop=(cc == 2))
                o1 = ps.tile([128, G], f32, name=f"o1_{st}", tag="sc",
                             bufs=2)
                for cc in range(3):
                    nc.tensor.matmul(
                        o1, gt[cc][:, 128 * st:128 * (st + 1)], wov(cc, 1),
                        start=(cc == 0), stop=(cc == 2))
                ob = outp.tile([128, D], f16, name=f"ob{st}", tag="ob",
                               bufs=4)
                nc.vector.tensor_copy(out=ob[:, 0:G], in_=o0)
                if st >= 12:  # tail: evict via ACT+DVE in parallel
                    nc.scalar.activation(
                        out=ob[:, G:D], in_=o1,
                        func=mybir.ActivationFunctionType.Copy)
                else:
                    nc.vector.tensor_copy(out=ob[:, G:D], in_=o1)
                nc.gpsimd.dma_start(
                    out=y_d[128 * st:128 * (st + 1), :], in_=ob)

            # ---- attention: one global software pipeline over all
            # (sc, pair, jb) blocks so neither the PE nor ACT drains at
            # window or pair boundaries ----
            def scores_exp(pair, sc, jb):
                col0 = max(0, 128 * jb - W * sc)
                sct = ps.tile([128, 1024], f32, name=f"sc{pair}_{sc}_{jb}",
                              tag="sc", bufs=2)
                nc.tensor.matmul(
                    sct[:, col0:W],
                    kt[pair][0:64, 128 * jb:128 * (jb + 1)],
                    qt[pair][0:64, W * sc + col0:W * (sc + 1)],
                    start=True, stop=True)
                nc.tensor.matmul(
                    sct[:, W:2 * W - col0],
                    kt[pair][64:128, 128 * jb:128 * (jb + 1)],
                    qt[pair][64:128, W * sc + col0:W * (sc + 1)],
                    start=True, stop=True)
                ex = work.tile([128, 1024], f16, name=f"ex{pair}_{sc}_{jb}",
                               tag="exp", bufs=8)
                nc.scalar.activation(
                    out=ex[:, col0:2 * W - col0],
                    in_=sct[:, col0:2 * W - col0],
                    func=mybir.ActivationFunctionType.Exp,
                    scale=0.125 / (SC8 * SC8))
                if jb >= 4 * sc:  # zero the j>i triangle of the diag block
                    nc.gpsimd.tensor_mul(
                        ex[:, col0:col0 + 128], ex[:, col0:col0 + 128], mkv)
                    nc.gpsimd.tensor_mul(
                        ex[:, W:W + 128], ex[:, W:W + 128], mkv)
                return ex

            def finalize_copy(pair, sc, q0, q1, pv0, pv1):
                w = q1 - q0
                dnb = work.tile([128, w], f32, name=f"dn{pair}_{sc}_{q0}",
                                tag="dnb", bufs=2)
                nc.vector.tensor_copy(out=dnb[0:64, :],
                                      in_=pv0[64:128, q0:q1])
                nc.vector.tensor_copy(out=dnb[64:128, :],
                                      in_=pv1[64:128, q0:q1])
                return dnb

            def finalize_norm(pair, sc, q0, q1, pv0, pv1, dnb):
                # Normalize query cols [q0:q1) of this window into gt.
                # 1/dn as exp(-ln(dn)) on ScalarE: ln+exp share one
                # activation table set, so no table thrash, and the DVE
                # FIFO stays clear of the slow iterative reciprocal.
                w = q1 - q0
                cols = slice(W * sc + q0, W * sc + q1)
                rc = work.tile([128, w], f32, name=f"rc{pair}_{sc}_{q0}",
                               tag="rc", bufs=2)
                nc.scalar.activation(
                    out=rc, in_=dnb,
                    func=mybir.ActivationFunctionType.Ln)
                nc.scalar.activation(
                    out=rc, in_=rc,
                    func=mybir.ActivationFunctionType.Exp, scale=-1.0)
                nc.vector.tensor_mul(
                    gt[pair][0:64, cols], pv0[0:64, q0:q1], rc[0:64, :])
                nc.vector.tensor_mul(
                    gt[pair][64:128, cols], pv1[0:64, q0:q1], rc[64:128, :])

            LASTWIN = (NPAIR - 1, NSC - 1)

            def pv_dn(state):
                pv0, pv1, pair, sc, jb, ex = state
                col0 = max(0, 128 * jb - W * sc)
                first, last = (jb == 0), (jb == 4 * sc + 3)
                nc.tensor.matmul(
                    pv0[:, col0:W], vt[jb][:, 2 * pair, :],
                    ex[:, col0:W], start=first, stop=last)
                nc.tensor.matmul(
                    pv1[:, col0:W], vt[jb][:, 2 * pair + 1, :],
                    ex[:, W:2 * W - col0], start=first, stop=last)
                if (pair, sc) == LASTWIN and jb >= 4 * sc:
                    # last window: strip c of the diagonal is complete after
                    # block jb=4*sc+c (later blocks only write cols >=128*
                    # (c+1)), so normalize + out-project strip-by-strip to
                    # keep the PE busy through the tail.
                    c = jb - 4 * sc
                    dnb = finalize_copy(pair, sc, 128 * c, 128 * (c + 1),
                                        pv0, pv1)
                    finalize_norm(pair, sc, 128 * c, 128 * (c + 1),
                                  pv0, pv1, dnb)
                    outproj(4 * sc + c)
                elif last:
                    # copy dn out now (the boundary is a natural lull);
                    # defer ln/exp+muls ~2 blocks so the ACT FIFO doesn't
                    # idle-wait on the copies
                    dnb = finalize_copy(pair, sc, 0, W, pv0, pv1)
                    pending.append((gcur[0] + 2, lambda p=pair, s=sc,
                                    a=pv0, b=pv1, d=dnb:
                                    finalize_norm(p, s, 0, W, a, b, d)))

            # static filler plan: emit projection / out-proj units after
            # given global block indices (they're needed ~one round later
            # than emitted; DMA arrival order matches)
            fillers = {
                0: [lambda: proj_v(1)],
                1: [lambda: proj_qk_unit(1, 0)],
                2: [lambda: proj_v(2)],
                3: [lambda: proj_v(3)],
                4: [lambda: proj_qk_unit(2, 0)],
                6: [lambda: proj_qk_unit(0, 1)],
                8: [lambda: proj_qk_unit(1, 1)],
                10: [lambda: proj_qk_unit(2, 1)],
                12: [lambda: proj_v(4)],
                14: [lambda: proj_v(5)],
                16: [lambda: proj_v(6)],
                18: [lambda: proj_v(7)],
                20: [lambda: outproj(0)],
                22: [lambda: outproj(1)],
                24: [lambda: outproj(2)],
                26: [lambda: outproj(3)],
                28: [lambda: proj_qk_unit(0, 2)],
                30: [lambda: proj_qk_unit(1, 2)],
                32: [lambda: proj_qk_unit(2, 2)],
                36: [lambda: proj_v(8)],
                38: [lambda: proj_v(9)],
                40: [lambda: proj_v(10)],
                42: [lambda: proj_v(11)],
                45: [lambda: outproj(4)],
                48: [lambda: outproj(5)],
                51: [lambda: outproj(6)],
                54: [lambda: outproj(7)],
                57: [lambda: proj_qk_unit(0, 3)],
                60: [lambda: proj_qk_unit(1, 3)],
                63: [lambda: proj_qk_unit(2, 3)],
                66: [lambda: proj_v(12)],
                68: [lambda: proj_v(13)],
                70: [lambda: proj_v(14)],
                72: [lambda: proj_v(15)],
                75: [lambda: outproj(8)],
                79: [lambda: outproj(9)],
                83: [lambda: outproj(10)],
                87: [lambda: outproj(11)],
            }

            # HAM warm-up: dummy matmuls on the memset ones-strips into the
            # first window's pv tiles (overwritten by the real start=True
            # accumulation) keep the PE busy through the input-DMA wait so
            # the first projections run at 2.4GHz instead of 1.2.
            warm0 = ps.tile([128, W], f32, name="pv0_0_0", tag="apv",
                            bufs=2)
            warm1 = ps.tile([128, W], f32, name="pv1_0_0", tag="adn",
                            bufs=2)
            for i in range(14):
                dst = warm0 if i % 2 == 0 else warm1
                nc.tensor.matmul(
                    dst[0:64, 0:384], vt[i % 6][:, 0, 64:128],
                    vt[(i + 1) % 6][:, :, 64:128], start=True, stop=True)

            proj_qk_unit(0, 0)
            proj_v(0)

            prev = [None]
            pending = []
            windnb = {}
            gcur = [0]

            def block(pair, sc, jb, pv0, pv1):
                ex = scores_exp(pair, sc, jb)
                if prev[0] is not None:
                    pv_dn(prev[0])
                prev[0] = (pv0, pv1, pair, sc, jb, ex)

            for sc in range(NSC):
                for pair in range(NPAIR):
                    if (sc, pair) == (0, 0):
                        pv0, pv1 = warm0, warm1
                    else:
                        pv0 = ps.tile([128, W], f32,
                                      name=f"pv0_{pair}_{sc}",
                                      tag="apv", bufs=2)
                        pv1 = ps.tile([128, W], f32,
                                      name=f"pv1_{pair}_{sc}",
                                      tag="adn", bufs=2)
                    for jb in range(4 * sc + 4):
                        block(pair, sc, jb, pv0, pv1)
                        while pending and pending[0][0] <= gcur[0]:
                            pending.pop(0)[1]()
                        for fn in fillers.get(gcur[0], ()):
                            fn()
                        gcur[0] += 1
            pv_dn(prev[0])
            for _, fn in pending:
                fn()

    _split_waits(nc)
    return nc


def _get_program():
    global _PROGRAM
    if _PROGRAM is None:
        _PROGRAM = _build_program()
    return _PROGRAM


def _pack_chunks(wT, width):
    # [768, width] -> [128, 6*width] with chunk-major per-partition layout
    return np.ascontiguousarray(
        wT.reshape(NDC, 128, width).transpose(1, 0, 2).reshape(128, -1))


def kernel(x, Wq, Wk, Wv, Wo, bo):
    global LAST_RESULT
    from concourse.bass_utils import run_bass_kernel_spmd

    x = np.asarray(x, np.float32)
    Wq = np.asarray(Wq, np.float32)
    Wk = np.asarray(Wk, np.float32)
    Wv = np.asarray(Wv, np.float32)
    Wo = np.asarray(Wo, np.float32)
    bo = np.asarray(bo, np.float32)

    tri = np.tril(np.ones((128, 128), np.float32)).T  # 1 where j<=i
    mk = tri.astype(np.float16)

    in_maps = []
    for c in range(8):
        b, gi = divmod(c, 2)
        hs = slice(G * gi, G * (gi + 1))
        import ml_dtypes
        f8 = ml_dtypes.float8_e4m3

        xt = np.ascontiguousarray(x[b].T).astype(np.float16)
        xt3 = np.ascontiguousarray(
            xt.reshape(NDC, 128, NSC, W).transpose(1, 2, 0, 3))
        xt8 = xt3.astype(f8)
        wqT = Wq[hs, :].T.astype(np.float32)   # [768, 384]
        wkT = Wk[hs, :].T.astype(np.float32)
        wvT = Wv[hs, :].T.astype(np.float16)
        woT = Wo[:, hs].T.astype(np.float16)   # [384, 768]

        wqk8 = np.zeros((128, 2 * NPAIR, NDC, 128), f8)
        for pr in range(NPAIR):
            for t, wT in ((0, wqT), (1, wkT)):
                wqk8[:, 2 * pr + t] = (
                    wT[:, 128 * pr:128 * (pr + 1)] * SC8
                ).reshape(NDC, 128, 128).transpose(1, 0, 2).astype(f8)

        wts = np.concatenate([
            mk,
            _pack_chunks(wvT, G),
            np.ascontiguousarray(
                woT.reshape(3, 128, D).transpose(1, 0, 2).reshape(128, -1)),
        ], axis=1)
        assert wts.shape == (128, WX), wts.shape

        in_maps.append({
            "xt": np.ascontiguousarray(xt3),
            "xt8": xt8,
            "wqk8": np.ascontiguousarray(wqk8),
            "wts": np.ascontiguousarray(wts),
        })

    if PROFILE:
        _install_profile_hooks()
    nc = _get_program()
    res = run_bass_kernel_spmd(nc, in_maps, core_ids=list(range(8)),
                               trace=PROFILE, tmpdir=PROFILE_DIR)
    LAST_RESULT = res
    parts = [res.results[c]["y"] for c in range(8)]
    out = np.stack([parts[2 * b].astype(np.float32)
                    + parts[2 * b + 1].astype(np.float32)
                    + bo for b in range(B)])
    return out.astype(np.float32)


# revision 61
# speedup vs baseline: 1.0059x; 1.0059x over previous
"""Causal multi-head attention block (B=4, S=2048, D=768, H=12, Dh=64)
distributed over 8 NeuronCores: core = (batch, head-group), each core
computes its 6 heads end-to-end plus its partial output projection;
host sums the two partials per batch and adds the bias.

Self-contained: hardcodes all shapes; no sibling imports.
"""

import numpy as np

B, S, D = 4, 2048, 768
H, DH = 12, 64
G = 384          # channels per head group (6 heads)
NPAIR = 3        # head pairs per core
NSC = 4          # 512-wide query windows
W = 512
NST = 16         # 128-row s-tiles
NDC = 6          # 128-row D chunks

# packed-weights layout (per-partition column offsets in the wts tile)
MK0 = 0
WV0 = 128                    # full wv (6 chunks x 384)
WO0 = 2432                   # wo (3 chunks x 768)
WX = 4736
SC8 = 32.0                   # fp8 q/k weight pre-scale (avoids e4m3
                             # subnormals at w~0.02); undone in exp scale

_PROGRAM = None
PROFILE = False
PROFILE_DIR = None
LAST_RESULT = None


def _split_waits(nc, max_waits=1, max_updates=1):
    """This container's walrus rejects instructions carrying more than one
    semaphore wait/update ("Too many sync wait commands").  Move excess
    waits onto NoOps inserted before the owning instruction (same engine)
    and excess updates onto NoOps inserted after."""
    import concourse.mybir as mybir

    counter = [0]

    def nop(engine, waits, updates):
        counter[0] += 1
        n = mybir.InstNoOp(name=f"wsplit_nop_{counter[0]}", ins=[], outs=[])
        n.engine = engine
        n.sync_info = mybir.SyncInfo(on_wait=waits, on_update=updates)
        return n

    for bb in nc.main_func.blocks:
        out = []
        changed = False
        for ins in bb.instructions:
            si = ins.sync_info
            waits = list(si.on_wait) if si and si.on_wait else []
            updates = list(si.on_update) if si and si.on_update else []
            pre, post = [], []
            if len(waits) > max_waits:
                keep = waits[:max_waits - 1] if max_waits > 1 else []
                rest = waits[len(keep):]
                while rest:
                    chunk, rest = rest[:max_waits], rest[max_waits:]
                    pre.append(chunk)
                waits = keep
                changed = True
            if len(updates) > max_updates:
                rest = updates[max_updates:]
                updates = updates[:max_updates]
                while rest:
                    chunk, rest = rest[:max_updates], rest[max_updates:]
                    post.append(chunk)
                changed = True
            if pre or post:
                ins.sync_info = mybir.SyncInfo(
                    on_wait=waits, on_update=updates)
            for w in pre:
                out.append(nop(ins.engine, w, []))
            out.append(ins)
            for u in post:
                out.append(nop(ins.engine, [], u))
        if changed:
            bb.instructions = out


def _install_profile_hooks():
    """Dev-only (PROFILE=True): register the NTFF profile hook that the
    agent image's antenv lacks, and stub out the artifact upload."""
    import sys
    import types

    try:
        from antenv.axon_hooks import get_axon_ntff_profile_hook  # noqa: F401
    except ImportError:
        import antenv
        from trn_agent_boot import trn_boot

        hook = trn_boot._ntff_profile_via_ctypes("/opt/axon/libaxon_pjrt.so")
        mod = types.ModuleType("antenv.axon_hooks")
        mod._hook = hook
        mod.get_axon_ntff_profile_hook = lambda: mod._hook
        mod.set_axon_ntff_profile_hook = lambda h: setattr(mod, "_hook", h)
        sys.modules["antenv.axon_hooks"] = mod
        antenv.axon_hooks = mod

    from concourse import bass_utils

    bass_utils.upload_artifacts = lambda tmpdir: "local://" + tmpdir


def _build_program():
    import concourse.bass as bass
    import concourse.mybir as mybir
    import concourse.tile as tile

    f16 = mybir.dt.float16
    f32 = mybir.dt.float32
    f8 = mybir.dt.float8e4

    nc = bass.Bass()
    # xt is block-major: [128, window, chunk*512] so each window's slice is
    # one contiguous 6KB-per-partition DMA.  xt8/wqk8 are fp8 copies used
    # only by the DoubleRow q/k projections.
    xt_d = nc.declare_dram_parameter("xt", [128, NSC, NDC, W], f16,
                                     isOutput=False)
    xt8_d = nc.declare_dram_parameter("xt8", [128, NSC, NDC, W], f8,
                                      isOutput=False)
    wqk8_d = nc.declare_dram_parameter("wqk8", [128, 2 * NPAIR, NDC, 128],
                                       f8, isOutput=False)
    wts_d = nc.declare_dram_parameter("wts", [128, WX], f16, isOutput=False)
    y_d = nc.declare_dram_parameter("y", [S, D], f16, isOutput=True)

    with tile.TileContext(nc) as tc:
        with (
            tc.tile_pool(name="const", bufs=1) as const,
            tc.tile_pool(name="work", bufs=3) as work,
            tc.tile_pool(name="outp", bufs=3) as outp,
            tc.tile_pool(name="ps", bufs=2, space="PSUM") as ps,
        ):
            # ---- persistent SBUF tiles ----
            wts = const.tile([128, WX], f16, name="wts", tag="wts")
            xt = const.tile([128, NSC, NDC, W], f16, name="xt", tag="xt")
            xt8 = const.tile([128, NSC, NDC, W], f8, name="xt8", tag="xt8")
            wqk8 = const.tile([128, 2 * NPAIR, NDC, 128], f8, name="wqk8",
                              tag="wqk8")

            def xv(sc, dc, c0, c1):
                # chunk dc, window-relative cols c0..c1
                return xt[:, sc, dc, c0:c1]
            qt = [const.tile([128, S], f16, name=f"qt{p}", tag=f"qt{p}")
                  for p in range(NPAIR)]
            kt = [const.tile([128, S], f16, name=f"kt{p}", tag=f"kt{p}")
                  for p in range(NPAIR)]
            gt = [const.tile([128, S], f16, name=f"gt{p}", tag=f"gt{p}")
                  for p in range(NPAIR)]
            # vt[st]: per head h the 128 lhsT columns [v_h (64) | ones (64)]
            # so one matmul per head accumulates attn@V on out partitions
            # 0:64 and the softmax denominator (replicated) on 64:128.
            vt = [const.tile([128, 2 * NPAIR, 128], f16, name=f"vt{t}",
                             tag=f"vt{t}") for t in range(NST)]

            mkv = wts[:, MK0:MK0 + 128]

            def wvv(dc):
                return wts[:, WV0 + 384 * dc:WV0 + 384 * (dc + 1)]

            def wov(cc, half):
                b = WO0 + 768 * cc + 384 * half
                return wts[:, b:b + 384]

            # ---- input DMAs, need-ordered; both queues share one HBM
            # stream so the first-window deps (mk+pair0 qk, xt cols 0:512,
            # wv) go first and the rest rides behind compute ----
            # single queue, exact need order: the HBM stream is shared, so
            # interleaving a second queue only delays the critical set.
            # y-output DMAs ride the gpsimd queue instead.
            nc.sync.dma_start(out=wts[:, 0:WV0], in_=wts_d[:, 0:WV0])  # mk
            nc.sync.dma_start(out=wqk8[:, 0:2, :, :],
                              in_=wqk8_d[:, 0:2, :, :])
            nc.sync.dma_start(out=xt8[:, 0, :, :], in_=xt8_d[:, 0, :, :])
            nc.sync.dma_start(out=wts[:, WV0:WO0], in_=wts_d[:, WV0:WO0])
            nc.sync.dma_start(out=xt[:, 0, :, 0:128],
                              in_=xt_d[:, 0, :, 0:128])
            nc.sync.dma_start(out=xt[:, 0, :, 128:W],
                              in_=xt_d[:, 0, :, 128:W])
            nc.sync.dma_start(out=wqk8[:, 2:6, :, :],
                              in_=wqk8_d[:, 2:6, :, :])
            nc.sync.dma_start(out=xt8[:, 1, :, :], in_=xt8_d[:, 1, :, :])
            nc.sync.dma_start(out=xt[:, 1, :, :], in_=xt_d[:, 1, :, :])
            nc.sync.dma_start(out=xt8[:, 2, :, :], in_=xt8_d[:, 2, :, :])
            nc.sync.dma_start(out=xt[:, 2, :, :], in_=xt_d[:, 2, :, :])
            nc.sync.dma_start(out=xt8[:, 3, :, :], in_=xt8_d[:, 3, :, :])
            nc.sync.dma_start(out=xt[:, 3, :, :], in_=xt_d[:, 3, :, :])
            nc.sync.dma_start(out=wts[:, WO0:WX], in_=wts_d[:, WO0:WX])

            for st in range(NST):
                nc.vector.memset(vt[st][:, :, 64:128], 1.0)

            def proj_qk_unit(pair, sc):
                # fp8 DoubleRow: 3 matmuls of 2 packed 128-chunks each
                DR = mybir.MatmulPerfMode.DoubleRow
                qp = ps.tile([128, W], f32, name=f"qp{pair}_{sc}",
                             tag="sc", bufs=2)
                for g2 in range(3):
                    nc.tensor.matmul(
                        qp, wqk8[:, 2 * pair, 2 * g2:2 * g2 + 2, :],
                        xt8[:, sc, 2 * g2:2 * g2 + 2, :],
                        start=(g2 == 0), stop=(g2 == 2), perf_mode=DR)
                nc.vector.tensor_copy(
                    out=qt[pair][:, W * sc:W * (sc + 1)], in_=qp)
                kp = ps.tile([128, W], f32, name=f"kp{pair}_{sc}",
                             tag="sc", bufs=2)
                for g2 in range(3):
                    nc.tensor.matmul(
                        kp, wqk8[:, 2 * pair + 1, 2 * g2:2 * g2 + 2, :],
                        xt8[:, sc, 2 * g2:2 * g2 + 2, :],
                        start=(g2 == 0), stop=(g2 == 2), perf_mode=DR)
                nc.vector.tensor_copy(
                    out=kt[pair][:, W * sc:W * (sc + 1)], in_=kp)

            def proj_v(st):
                vp = ps.tile([128, 2 * NPAIR, 64], f32, name=f"vp{st}",
                             tag="sc", bufs=2)
                for dc in range(NDC):
                    nc.tensor.matmul(
                        vp,
                        xv(st // 4, dc, 128 * (st % 4), 128 * (st % 4 + 1)),
                        wvv(dc),
                        start=(dc == 0), stop=(dc == NDC - 1))
                nc.vector.tensor_copy(out=vt[st][:, :, 0:64], in_=vp)

            def outproj(st):
                o0 = ps.tile([128, G], f32, name=f"o0_{st}", tag="sc",
                             bufs=2)
                for cc in range(3):
                    nc.tensor.matmul(
                        o0, gt[cc][:, 128 * st:128 * (st + 1)], wov(cc, 0),
                        start=(cc == 0), stop=(cc == 2))
                o1 = ps.tile([128, G], f32, name=f"o1_{st}", tag="sc",
                             bufs=2)
                for cc in range(3):
                    nc.tensor.matmul(
                        o1, gt[cc][:, 128 * st:128 * (st + 1)], wov(cc, 1),
                        start=(cc == 0), stop=(cc == 2))
                ob = outp.tile([128, D], f16, name=f"ob{st}", tag="ob",
                               bufs=4)
                nc.vector.tensor_copy(out=ob[:, 0:G], in_=o0)
                if st >= 12:  # tail: evict via ACT+DVE in parallel
                    nc.scalar.activation(
                        out=ob[:, G:D], in_=o1,
                        func=mybir.ActivationFunctionType.Copy)
                else:
                    nc.vector.tensor_copy(out=ob[:, G:D], in_=o1)
                nc.gpsimd.dma_start(
                    out=y_d[128 * st:128 * (st + 1), :], in_=ob)

            # ---- attention: one global software pipeline over all
            # (sc, pair, jb) blocks so neither the PE nor ACT drains at
            # window or pair boundaries ----
            def scores_exp(pair, sc, jb):
                col0 = max(0, 128 * jb - W * sc)
                sct = ps.tile([128, 1024], f32, name=f"sc{pair}_{sc}_{jb}",
                              tag="sc", bufs=2)
                nc.tensor.matmul(
                    sct[:, col0:W],
                    kt[pair][0:64, 128 * jb:128 * (jb + 1)],
                    qt[pair][0:64, W * sc + col0:W * (sc + 1)],
                    start=True, stop=True)
                nc.tensor.matmul(
                    sct[:, W:2 * W - col0],
                    kt[pair][64:128, 128 * jb:128 * (jb + 1)],
                    qt[pair][64:128, W * sc + col0:W * (sc + 1)],
                    start=True, stop=True)
                ex = work.tile([128, 1024], f16, name=f"ex{pair}_{sc}_{jb}",
                               tag="exp", bufs=6)
                nc.scalar.activation(
                    out=ex[:, col0:2 * W - col0],
                    in_=sct[:, col0:2 * W - col0],
                    func=mybir.ActivationFunctionType.Exp,
                    scale=0.125 / (SC8 * SC8))
                if jb >= 4 * sc:  # zero the j>i triangle of the diag block
                    nc.gpsimd.tensor_mul(
                        ex[:, col0:col0 + 128], ex[:, col0:col0 + 128], mkv)
                    nc.gpsimd.tensor_mul(
                        ex[:, W:W + 128], ex[:, W:W + 128], mkv)
                return ex

            def finalize_copy(pair, sc, q0, q1, pv0, pv1):
                w = q1 - q0
                dnb = work.tile([128, w], f32, name=f"dn{pair}_{sc}_{q0}",
                                tag="dnb", bufs=2)
                nc.vector.tensor_copy(out=dnb[0:64, :],
                                      in_=pv0[64:128, q0:q1])
                nc.vector.tensor_copy(out=dnb[64:128, :],
                                      in_=pv1[64:128, q0:q1])
                return dnb

            def finalize_norm(pair, sc, q0, q1, pv0, pv1, dnb):
                # Normalize query cols [q0:q1) of this window into gt.
                # 1/dn as exp(-ln(dn)) on ScalarE: ln+exp share one
                # activation table set, so no table thrash, and the DVE
                # FIFO stays clear of the slow iterative reciprocal.
                w = q1 - q0
                cols = slice(W * sc + q0, W * sc + q1)
                rc = work.tile([128, w], f32, name=f"rc{pair}_{sc}_{q0}",
                               tag="rc", bufs=2)
                nc.scalar.activation(
                    out=rc, in_=dnb,
                    func=mybir.ActivationFunctionType.Ln)
                nc.scalar.activation(
                    out=rc, in_=rc,
                    func=mybir.ActivationFunctionType.Exp, scale=-1.0)
                nc.vector.tensor_mul(
                    gt[pair][0:64, cols], pv0[0:64, q0:q1], rc[0:64, :])
                nc.vector.tensor_mul(
                    gt[pair][64:128, cols], pv1[0:64, q0:q1], rc[64:128, :])

            LASTWIN = (NPAIR - 1, NSC - 1)

            def pv_dn(state):
                pv0, pv1, pair, sc, jb, ex = state
                col0 = max(0, 128 * jb - W * sc)
                first, last = (jb == 0), (jb == 4 * sc + 3)
                nc.tensor.matmul(
                    pv0[:, col0:W], vt[jb][:, 2 * pair, :],
                    ex[:, col0:W], start=first, stop=last)
                nc.tensor.matmul(
                    pv1[:, col0:W], vt[jb][:, 2 * pair + 1, :],
                    ex[:, W:2 * W - col0], start=first, stop=last)
                if (pair, sc) == LASTWIN and jb >= 4 * sc:
                    # last window: strip c of the diagonal is complete after
                    # block jb=4*sc+c (later blocks only write cols >=128*
                    # (c+1)), so normalize + out-project strip-by-strip to
                    # keep the PE busy through the tail.
                    c = jb - 4 * sc
                    dnb = finalize_copy(pair, sc, 128 * c, 128 * (c + 1),
                                        pv0, pv1)
                    finalize_norm(pair, sc, 128 * c, 128 * (c + 1),
                                  pv0, pv1, dnb)
                    outproj(4 * sc + c)
                elif last:
                    # copy dn out now (the boundary is a natural lull);
                    # defer ln/exp+muls ~2 blocks so the ACT FIFO doesn't
                    # idle-wait on the copies
                    dnb = finalize_copy(pair, sc, 0, W, pv0, pv1)
                    pending.append((gcur[0] + 2, lambda p=pair, s=sc,
                                    a=pv0, b=pv1, d=dnb:
                                    finalize_norm(p, s, 0, W, a, b, d)))

            # static filler plan: emit projection / out-proj units after
            # given global block indices (they're needed ~one round later
            # than emitted; DMA arrival order matches)
            fillers = {
                0: [lambda: proj_v(1)],
                1: [lambda: proj_qk_unit(1, 0)],
                2: [lambda: proj_v(2)],
                3: [lambda: proj_v(3)],
                4: [lambda: proj_qk_unit(2, 0)],
                6: [lambda: proj_qk_unit(0, 1)],
                8: [lambda: proj_qk_unit(1, 1)],
                10: [lambda: proj_qk_unit(2, 1)],
                12: [lambda: proj_v(4)],
                14: [lambda: proj_v(5)],
                16: [lambda: proj_v(6)],
                18: [lambda: proj_v(7)],
                20: [lambda: outproj(0)],
                22: [lambda: outproj(1)],
                24: [lambda: outproj(2)],
                26: [lambda: outproj(3)],
                28: [lambda: proj_qk_unit(0, 2)],
                30: [lambda: proj_qk_unit(1, 2)],
                32: [lambda: proj_qk_unit(2, 2)],
                36: [lambda: proj_v(8)],
                38: [lambda: proj_v(9)],
                40: [lambda: proj_v(10)],
                42: [lambda: proj_v(11)],
                45: [lambda: outproj(4)],
                48: [lambda: outproj(5)],
                51: [lambda: outproj(6)],
                54: [lambda: outproj(7)],
                57: [lambda: proj_qk_unit(0, 3)],
                60: [lambda: proj_qk_unit(1, 3)],
                63: [lambda: proj_qk_unit(2, 3)],
                66: [lambda: proj_v(12)],
                68: [lambda: proj_v(13)],
                70: [lambda: proj_v(14)],
                72: [lambda: proj_v(15)],
                75: [lambda: outproj(8)],
                79: [lambda: outproj(9)],
                83: [lambda: outproj(10)],
                87: [lambda: outproj(11)],
            }

            # HAM warm-up: dummy matmuls on the memset ones-strips into the
            # first window's pv tiles (overwritten by the real start=True
            # accumulation) keep the PE busy through the input-DMA wait so
            # the first projections run at 2.4GHz instead of 1.2.
            warm0 = ps.tile([128, W], f32, name="pv0_0_0", tag="apv",
                            bufs=2)
            warm1 = ps.tile([128, W], f32, name="pv1_0_0", tag="adn",
                            bufs=2)
            for i in range(22):
                dst = warm0 if i % 2 == 0 else warm1
                nc.tensor.matmul(
                    dst[0:64, 0:384], vt[i % 6][:, 0, 64:128],
                    vt[(i + 1) % 6][:, :, 64:128], start=True, stop=True)

            proj_qk_unit(0, 0)
            proj_v(0)

            prev = [None]
            pending = []
            windnb = {}
            gcur = [0]

            def block(pair, sc, jb, pv0, pv1):
                ex = scores_exp(pair, sc, jb)
                if prev[0] is not None:
                    pv_dn(prev[0])
                prev[0] = (pv0, pv1, pair, sc, jb, ex)

            for sc in range(NSC):
                for pair in range(NPAIR):
                    if (sc, pair) == (0, 0):
                        pv0, pv1 = warm0, warm1
                    else:
                        pv0 = ps.tile([128, W], f32,
                                      name=f"pv0_{pair}_{sc}",
                                      tag="apv", bufs=2)
                        pv1 = ps.tile([128, W], f32,
                                      name=f"pv1_{pair}_{sc}",
                                      tag="adn", bufs=2)
                    for jb in range(4 * sc + 4):
                        block(pair, sc, jb, pv0, pv1)
                        while pending and pending[0][0] <= gcur[0]:
                            pending.pop(0)[1]()
                        for fn in fillers.get(gcur[0], ()):
                            fn()
                        gcur[0] += 1
            pv_dn(prev[0])
            for _, fn in pending:
                fn()

    _split_waits(nc)
    return nc


def _get_program():
    global _PROGRAM
    if _PROGRAM is None:
        _PROGRAM = _build_program()
    return _PROGRAM


def _pack_chunks(wT, width):
    # [768, width] -> [128, 6*width] with chunk-major per-partition layout
    return np.ascontiguousarray(
        wT.reshape(NDC, 128, width).transpose(1, 0, 2).reshape(128, -1))


def kernel(x, Wq, Wk, Wv, Wo, bo):
    global LAST_RESULT
    from concourse.bass_utils import run_bass_kernel_spmd

    x = np.asarray(x, np.float32)
    Wq = np.asarray(Wq, np.float32)
    Wk = np.asarray(Wk, np.float32)
    Wv = np.asarray(Wv, np.float32)
    Wo = np.asarray(Wo, np.float32)
    bo = np.asarray(bo, np.float32)

    tri = np.tril(np.ones((128, 128), np.float32)).T  # 1 where j<=i
    mk = tri.astype(np.float16)

    in_maps = []
    for c in range(8):
        b, gi = divmod(c, 2)
        hs = slice(G * gi, G * (gi + 1))
        import ml_dtypes
        f8 = ml_dtypes.float8_e4m3

        xt = np.ascontiguousarray(x[b].T).astype(np.float16)
        xt3 = np.ascontiguousarray(
            xt.reshape(NDC, 128, NSC, W).transpose(1, 2, 0, 3))
        xt8 = xt3.astype(f8)
        wqT = Wq[hs, :].T.astype(np.float32)   # [768, 384]
        wkT = Wk[hs, :].T.astype(np.float32)
        wvT = Wv[hs, :].T.astype(np.float16)
        woT = Wo[:, hs].T.astype(np.float16)   # [384, 768]

        wqk8 = np.zeros((128, 2 * NPAIR, NDC, 128), f8)
        for pr in range(NPAIR):
            for t, wT in ((0, wqT), (1, wkT)):
                wqk8[:, 2 * pr + t] = (
                    wT[:, 128 * pr:128 * (pr + 1)] * SC8
                ).reshape(NDC, 128, 128).transpose(1, 0, 2).astype(f8)

        wts = np.concatenate([
            mk,
            _pack_chunks(wvT, G),
            np.ascontiguousarray(
                woT.reshape(3, 128, D).transpose(1, 0, 2).reshape(128, -1)),
        ], axis=1)
        assert wts.shape == (128, WX), wts.shape

        in_maps.append({
            "xt": np.ascontiguousarray(xt3),
            "xt8": xt8,
            "wqk8": np.ascontiguousarray(wqk8),
            "wts": np.ascontiguousarray(wts),
        })

    if PROFILE:
        _install_profile_hooks()
    nc = _get_program()
    res = run_bass_kernel_spmd(nc, in_maps, core_ids=list(range(8)),
                               trace=PROFILE, tmpdir=PROFILE_DIR)
    LAST_RESULT = res
    parts = [res.results[c]["y"] for c in range(8)]
    out = np.stack([parts[2 * b].astype(np.float32)
                    + parts[2 * b + 1].astype(np.float32)
                    + bo for b in range(B)])
    return out.astype(np.float32)


# revision 63
# speedup vs baseline: 1.0294x; 1.0233x over previous
"""Causal multi-head attention block (B=4, S=2048, D=768, H=12, Dh=64)
distributed over 8 NeuronCores: core = (batch, head-group), each core
computes its 6 heads end-to-end plus its partial output projection;
host sums the two partials per batch and adds the bias.

Self-contained: hardcodes all shapes; no sibling imports.
"""

import numpy as np

B, S, D = 4, 2048, 768
H, DH = 12, 64
G = 384          # channels per head group (6 heads)
NPAIR = 3        # head pairs per core
NSC = 4          # 512-wide query windows
W = 512
NST = 16         # 128-row s-tiles
NDC = 6          # 128-row D chunks

# packed-weights layout (per-partition column offsets in the wts tile)
MK0 = 0
WV0 = 128                    # full wv (6 chunks x 384)
WO0 = 2432                   # wo (3 chunks x 768)
WX = 4736
SC8 = 32.0                   # fp8 q/k weight pre-scale (avoids e4m3
                             # subnormals at w~0.02); undone in exp scale

_PROGRAM = None
PROFILE = False
PROFILE_DIR = None
LAST_RESULT = None


def _split_waits(nc, max_waits=1, max_updates=1):
    """This container's walrus rejects instructions carrying more than one
    semaphore wait/update ("Too many sync wait commands").  Move excess
    waits onto NoOps inserted before the owning instruction (same engine)
    and excess updates onto NoOps inserted after."""
    import concourse.mybir as mybir

    counter = [0]

    def nop(engine, waits, updates):
        counter[0] += 1
        n = mybir.InstNoOp(name=f"wsplit_nop_{counter[0]}", ins=[], outs=[])
        n.engine = engine
        n.sync_info = mybir.SyncInfo(on_wait=waits, on_update=updates)
        return n

    for bb in nc.main_func.blocks:
        out = []
        changed = False
        for ins in bb.instructions:
            si = ins.sync_info
            waits = list(si.on_wait) if si and si.on_wait else []
            updates = list(si.on_update) if si and si.on_update else []
            pre, post = [], []
            if len(waits) > max_waits:
                keep = waits[:max_waits - 1] if max_waits > 1 else []
                rest = waits[len(keep):]
                while rest:
                    chunk, rest = rest[:max_waits], rest[max_waits:]
                    pre.append(chunk)
                waits = keep
                changed = True
            if len(updates) > max_updates:
                rest = updates[max_updates:]
                updates = updates[:max_updates]
                while rest:
                    chunk, rest = rest[:max_updates], rest[max_updates:]
                    post.append(chunk)
                changed = True
            if pre or post:
                ins.sync_info = mybir.SyncInfo(
                    on_wait=waits, on_update=updates)
            for w in pre:
                out.append(nop(ins.engine, w, []))
            out.append(ins)
            for u in post:
                out.append(nop(ins.engine, [], u))
        if changed:
            bb.instructions = out


def _install_profile_hooks():
    """Dev-only (PROFILE=True): register the NTFF profile hook that the
    agent image's antenv lacks, and stub out the artifact upload."""
    import sys
    import types

    try:
        from antenv.axon_hooks import get_axon_ntff_profile_hook  # noqa: F401
    except ImportError:
        import antenv
        from trn_agent_boot import trn_boot

        hook = trn_boot._ntff_profile_via_ctypes("/opt/axon/libaxon_pjrt.so")
        mod = types.ModuleType("antenv.axon_hooks")
        mod._hook = hook
        mod.get_axon_ntff_profile_hook = lambda: mod._hook
        mod.set_axon_ntff_profile_hook = lambda h: setattr(mod, "_hook", h)
        sys.modules["antenv.axon_hooks"] = mod
        antenv.axon_hooks = mod

    from concourse import bass_utils

    bass_utils.upload_artifacts = lambda tmpdir: "local://" + tmpdir


def _build_program():
    import concourse.bass as bass
    import concourse.mybir as mybir
    import concourse.tile as tile

    f16 = mybir.dt.float16
    f32 = mybir.dt.float32
    f8 = mybir.dt.float8e4

    nc = bass.Bass()
    # xt is block-major: [128, window, chunk*512] so each window's slice is
    # one contiguous 6KB-per-partition DMA.  xt8/wqk8 are fp8 copies used
    # only by the DoubleRow q/k projections.
    xt_d = nc.declare_dram_parameter("xt", [128, NSC, NDC, W], f16,
                                     isOutput=False)
    xt8_d = nc.declare_dram_parameter("xt8", [128, NSC, NDC, W], f8,
                                      isOutput=False)
    wqk8_d = nc.declare_dram_parameter("wqk8", [128, 2 * NPAIR, NDC, 128],
                                       f8, isOutput=False)
    wts_d = nc.declare_dram_parameter("wts", [128, WX], f16, isOutput=False)
    y_d = nc.declare_dram_parameter("y", [S, D], f16, isOutput=True)

    with tile.TileContext(nc) as tc:
        with (
            tc.tile_pool(name="const", bufs=1) as const,
            tc.tile_pool(name="work", bufs=3) as work,
            tc.tile_pool(name="outp", bufs=3) as outp,
            tc.tile_pool(name="ps", bufs=2, space="PSUM") as ps,
        ):
            # ---- persistent SBUF tiles ----
            wts = const.tile([128, WX], f16, name="wts", tag="wts")
            xt = const.tile([128, NSC, NDC, W], f16, name="xt", tag="xt")
            xt8 = const.tile([128, NSC, NDC, W], f8, name="xt8", tag="xt8")
            wqk8 = const.tile([128, 2 * NPAIR, NDC, 128], f8, name="wqk8",
                              tag="wqk8")

            def xv(sc, dc, c0, c1):
                # chunk dc, window-relative cols c0..c1
                return xt[:, sc, dc, c0:c1]
            qt = [const.tile([128, S], f16, name=f"qt{p}", tag=f"qt{p}")
                  for p in range(NPAIR)]
            kt = [const.tile([128, S], f16, name=f"kt{p}", tag=f"kt{p}")
                  for p in range(NPAIR)]
            gt = [const.tile([128, S], f16, name=f"gt{p}", tag=f"gt{p}")
                  for p in range(NPAIR)]
            # vt[st]: per head h the 128 lhsT columns [v_h (64) | ones (64)]
            # so one matmul per head accumulates attn@V on out partitions
            # 0:64 and the softmax denominator (replicated) on 64:128.
            vt = [const.tile([128, 2 * NPAIR, 128], f16, name=f"vt{t}",
                             tag=f"vt{t}") for t in range(NST)]

            mkv = wts[:, MK0:MK0 + 128]

            def wvv(dc):
                return wts[:, WV0 + 384 * dc:WV0 + 384 * (dc + 1)]

            def wov(cc, half):
                b = WO0 + 768 * cc + 384 * half
                return wts[:, b:b + 384]

            # ---- input DMAs, need-ordered; both queues share one HBM
            # stream so the first-window deps (mk+pair0 qk, xt cols 0:512,
            # wv) go first and the rest rides behind compute ----
            # single queue, exact need order: the HBM stream is shared, so
            # interleaving a second queue only delays the critical set.
            # y-output DMAs ride the gpsimd queue instead.
            nc.sync.dma_start(out=wts[:, 0:WV0], in_=wts_d[:, 0:WV0])  # mk
            nc.sync.dma_start(out=wqk8[:, 0:2, :, :],
                              in_=wqk8_d[:, 0:2, :, :])
            nc.sync.dma_start(out=xt8[:, 0, :, :], in_=xt8_d[:, 0, :, :])
            nc.sync.dma_start(out=wts[:, WV0:WO0], in_=wts_d[:, WV0:WO0])
            nc.sync.dma_start(out=xt[:, 0, :, 0:128],
                              in_=xt_d[:, 0, :, 0:128])
            nc.sync.dma_start(out=xt[:, 0, :, 128:W],
                              in_=xt_d[:, 0, :, 128:W])
            nc.sync.dma_start(out=wqk8[:, 2:6, :, :],
                              in_=wqk8_d[:, 2:6, :, :])
            nc.sync.dma_start(out=xt8[:, 1, :, :], in_=xt8_d[:, 1, :, :])
            nc.sync.dma_start(out=xt[:, 1, :, :], in_=xt_d[:, 1, :, :])
            nc.sync.dma_start(out=xt8[:, 2, :, :], in_=xt8_d[:, 2, :, :])
            nc.sync.dma_start(out=xt[:, 2, :, :], in_=xt_d[:, 2, :, :])
            nc.sync.dma_start(out=xt8[:, 3, :, :], in_=xt8_d[:, 3, :, :])
            nc.sync.dma_start(out=xt[:, 3, :, :], in_=xt_d[:, 3, :, :])
            nc.sync.dma_start(out=wts[:, WO0:WX], in_=wts_d[:, WO0:WX])

            for st in range(NST):
                nc.vector.memset(vt[st][:, :, 64:128], 1.0)

            def proj_qk_unit(pair, sc):
                # fp8 DoubleRow: 3 matmuls of 2 packed 128-chunks each
                DR = mybir.MatmulPerfMode.DoubleRow
                qp = ps.tile([128, W], f32, name=f"qp{pair}_{sc}",
                             tag="sc", bufs=2)
                for g2 in range(3):
                    nc.tensor.matmul(
                        qp, wqk8[:, 2 * pair, 2 * g2:2 * g2 + 2, :],
                        xt8[:, sc, 2 * g2:2 * g2 + 2, :],
                        start=(g2 == 0), stop=(g2 == 2), perf_mode=DR)
                nc.vector.tensor_copy(
                    out=qt[pair][:, W * sc:W * (sc + 1)], in_=qp)
                kp = ps.tile([128, W], f32, name=f"kp{pair}_{sc}",
                             tag="sc", bufs=2)
                for g2 in range(3):
                    nc.tensor.matmul(
                        kp, wqk8[:, 2 * pair + 1, 2 * g2:2 * g2 + 2, :],
                        xt8[:, sc, 2 * g2:2 * g2 + 2, :],
                        start=(g2 == 0), stop=(g2 == 2), perf_mode=DR)
                nc.vector.tensor_copy(
                    out=kt[pair][:, W * sc:W * (sc + 1)], in_=kp)

            def proj_v(st):
                vp = ps.tile([128, 2 * NPAIR, 64], f32, name=f"vp{st}",
                             tag="sc", bufs=2)
                for dc in range(NDC):
                    nc.tensor.matmul(
                        vp,
                        xv(st // 4, dc, 128 * (st % 4), 128 * (st % 4 + 1)),
                        wvv(dc),
                        start=(dc == 0), stop=(dc == NDC - 1))
                nc.vector.tensor_copy(out=vt[st][:, :, 0:64], in_=vp)

            def outproj(st):
                o0 = ps.tile([128, G], f32, name=f"o0_{st}", tag="sc",
                             bufs=2)
                for cc in range(3):
                    nc.tensor.matmul(
                        o0, gt[cc][:, 128 * st:128 * (st + 1)], wov(cc, 0),
                        start=(cc == 0), stop=(cc == 2))
                o1 = ps.tile([128, G], f32, name=f"o1_{st}", tag="sc",
                             bufs=2)
                for cc in range(3):
                    nc.tensor.matmul(
                        o1, gt[cc][:, 128 * st:128 * (st + 1)], wov(cc, 1),
                        start=(cc == 0), stop=(cc == 2))
                ob = outp.tile([128, D], f16, name=f"ob{st}", tag="ob",
                               bufs=4)
                nc.vector.tensor_copy(out=ob[:, 0:G], in_=o0)
                if st >= 12:  # tail: evict via ACT+DVE in parallel
                    nc.scalar.activation(
                        out=ob[:, G:D], in_=o1,
                        func=mybir.ActivationFunctionType.Copy)
                else:
                    nc.vector.tensor_copy(out=ob[:, G:D], in_=o1)
                if 12 <= st < 15:
                    # keep HAM warm across the tail's inter-strip waits:
                    # dummy matmuls into the just-consumed o0 (WAR-safe,
                    # never read) run during the dependency stall so the
                    # next strip's matmuls stay at 2.4GHz.
                    for i in range(4):
                        nc.tensor.matmul(
                            o0[0:64, 0:G], vt[i][:, 0, 64:128],
                            vt[i + 1][:, :, 64:128], start=True, stop=True)
                nc.gpsimd.dma_start(
                    out=y_d[128 * st:128 * (st + 1), :], in_=ob)

            # ---- attention: one global software pipeline over all
            # (sc, pair, jb) blocks so neither the PE nor ACT drains at
            # window or pair boundaries ----
            def scores_exp(pair, sc, jb):
                col0 = max(0, 128 * jb - W * sc)
                sct = ps.tile([128, 1024], f32, name=f"sc{pair}_{sc}_{jb}",
                              tag="sc", bufs=2)
                nc.tensor.matmul(
                    sct[:, col0:W],
                    kt[pair][0:64, 128 * jb:128 * (jb + 1)],
                    qt[pair][0:64, W * sc + col0:W * (sc + 1)],
                    start=True, stop=True)
                nc.tensor.matmul(
                    sct[:, W:2 * W - col0],
                    kt[pair][64:128, 128 * jb:128 * (jb + 1)],
                    qt[pair][64:128, W * sc + col0:W * (sc + 1)],
                    start=True, stop=True)
                ex = work.tile([128, 1024], f16, name=f"ex{pair}_{sc}_{jb}",
                               tag="exp", bufs=6)
                nc.scalar.activation(
                    out=ex[:, col0:2 * W - col0],
                    in_=sct[:, col0:2 * W - col0],
                    func=mybir.ActivationFunctionType.Exp,
                    scale=0.125 / (SC8 * SC8))
                if jb >= 4 * sc:  # zero the j>i triangle of the diag block
                    nc.gpsimd.tensor_mul(
                        ex[:, col0:col0 + 128], ex[:, col0:col0 + 128], mkv)
                    nc.gpsimd.tensor_mul(
                        ex[:, W:W + 128], ex[:, W:W + 128], mkv)
                return ex

            def finalize_copy(pair, sc, q0, q1, pv0, pv1):
                w = q1 - q0
                dnb = work.tile([128, w], f32, name=f"dn{pair}_{sc}_{q0}",
                                tag="dnb", bufs=2)
                nc.vector.tensor_copy(out=dnb[0:64, :],
                                      in_=pv0[64:128, q0:q1])
                nc.vector.tensor_copy(out=dnb[64:128, :],
                                      in_=pv1[64:128, q0:q1])
                return dnb

            def finalize_norm(pair, sc, q0, q1, pv0, pv1, dnb):
                # Normalize query cols [q0:q1) of this window into gt.
                # 1/dn as exp(-ln(dn)) on ScalarE: ln+exp share one
                # activation table set, so no table thrash, and the DVE
                # FIFO stays clear of the slow iterative reciprocal.
                w = q1 - q0
                cols = slice(W * sc + q0, W * sc + q1)
                rc = work.tile([128, w], f32, name=f"rc{pair}_{sc}_{q0}",
                               tag="rc", bufs=2)
                nc.scalar.activation(
                    out=rc, in_=dnb,
                    func=mybir.ActivationFunctionType.Ln)
                nc.scalar.activation(
                    out=rc, in_=rc,
                    func=mybir.ActivationFunctionType.Exp, scale=-1.0)
                nc.vector.tensor_mul(
                    gt[pair][0:64, cols], pv0[0:64, q0:q1], rc[0:64, :])
                nc.vector.tensor_mul(
                    gt[pair][64:128, cols], pv1[0:64, q0:q1], rc[64:128, :])

            LASTWIN = (NPAIR - 1, NSC - 1)

            def pv_dn(state):
                pv0, pv1, pair, sc, jb, ex = state
                col0 = max(0, 128 * jb - W * sc)
                first, last = (jb == 0), (jb == 4 * sc + 3)
                nc.tensor.matmul(
                    pv0[:, col0:W], vt[jb][:, 2 * pair, :],
                    ex[:, col0:W], start=first, stop=last)
                nc.tensor.matmul(
                    pv1[:, col0:W], vt[jb][:, 2 * pair + 1, :],
                    ex[:, W:2 * W - col0], start=first, stop=last)
                if (pair, sc) == LASTWIN and jb >= 4 * sc:
                    # last window: strip c of the diagonal is complete after
                    # block jb=4*sc+c (later blocks only write cols >=128*
                    # (c+1)), so normalize + out-project strip-by-strip to
                    # keep the PE busy through the tail.
                    c = jb - 4 * sc
                    dnb = finalize_copy(pair, sc, 128 * c, 128 * (c + 1),
                                        pv0, pv1)
                    finalize_norm(pair, sc, 128 * c, 128 * (c + 1),
                                  pv0, pv1, dnb)
                    outproj(4 * sc + c)
                elif last:
                    # copy dn out now (the boundary is a natural lull);
                    # defer ln/exp+muls ~2 blocks so the ACT FIFO doesn't
                    # idle-wait on the copies
                    dnb = finalize_copy(pair, sc, 0, W, pv0, pv1)
                    pending.append((gcur[0] + 2, lambda p=pair, s=sc,
                                    a=pv0, b=pv1, d=dnb:
                                    finalize_norm(p, s, 0, W, a, b, d)))

            # static filler plan: emit projection / out-proj units after
            # given global block indices (they're needed ~one round later
            # than emitted; DMA arrival order matches)
            fillers = {
                0: [lambda: proj_v(1)],
                1: [lambda: proj_qk_unit(1, 0)],
                2: [lambda: proj_v(2)],
                3: [lambda: proj_v(3)],
                4: [lambda: proj_qk_unit(2, 0)],
                6: [lambda: proj_qk_unit(0, 1)],
                8: [lambda: proj_qk_unit(1, 1)],
                10: [lambda: proj_qk_unit(2, 1)],
                12: [lambda: proj_v(4)],
                14: [lambda: proj_v(5)],
                16: [lambda: proj_v(6)],
                18: [lambda: proj_v(7)],
                20: [lambda: outproj(0)],
                22: [lambda: outproj(1)],
                24: [lambda: outproj(2)],
                26: [lambda: outproj(3)],
                28: [lambda: proj_qk_unit(0, 2)],
                30: [lambda: proj_qk_unit(1, 2)],
                32: [lambda: proj_qk_unit(2, 2)],
                36: [lambda: proj_v(8)],
                38: [lambda: proj_v(9)],
                40: [lambda: proj_v(10)],
                42: [lambda: proj_v(11)],
                45: [lambda: outproj(4)],
                48: [lambda: outproj(5)],
                51: [lambda: outproj(6)],
                54: [lambda: outproj(7)],
                57: [lambda: proj_qk_unit(0, 3)],
                60: [lambda: proj_qk_unit(1, 3)],
                63: [lambda: proj_qk_unit(2, 3)],
                66: [lambda: proj_v(12)],
                68: [lambda: proj_v(13)],
                70: [lambda: proj_v(14)],
                72: [lambda: proj_v(15)],
                75: [lambda: outproj(8)],
                79: [lambda: outproj(9)],
                83: [lambda: outproj(10)],
                87: [lambda: outproj(11)],
            }

            # HAM warm-up: dummy matmuls on the memset ones-strips into the
            # first window's pv tiles (overwritten by the real start=True
            # accumulation) keep the PE busy through the input-DMA wait so
            # the first projections run at 2.4GHz instead of 1.2.
            warm0 = ps.tile([128, W], f32, name="pv0_0_0", tag="apv",
                            bufs=2)
            warm1 = ps.tile([128, W], f32, name="pv1_0_0", tag="adn",
                            bufs=2)
            for i in range(14):
                dst = warm0 if i % 2 == 0 else warm1
                nc.tensor.matmul(
                    dst[0:64, 0:384], vt[i % 6][:, 0, 64:128],
                    vt[(i + 1) % 6][:, :, 64:128], start=True, stop=True)

            proj_qk_unit(0, 0)
            proj_v(0)

            prev = [None]
            pending = []
            windnb = {}
            gcur = [0]

            def block(pair, sc, jb, pv0, pv1):
                ex = scores_exp(pair, sc, jb)
                if prev[0] is not None:
                    pv_dn(prev[0])
                prev[0] = (pv0, pv1, pair, sc, jb, ex)

            for sc in range(NSC):
                for pair in range(NPAIR):
                    if (sc, pair) == (0, 0):
                        pv0, pv1 = warm0, warm1
                    else:
                        pv0 = ps.tile([128, W], f32,
                                      name=f"pv0_{pair}_{sc}",
                                      tag="apv", bufs=2)
                        pv1 = ps.tile([128, W], f32,
                                      name=f"pv1_{pair}_{sc}",
                                      tag="adn", bufs=2)
                    for jb in range(4 * sc + 4):
                        block(pair, sc, jb, pv0, pv1)
                        while pending and pending[0][0] <= gcur[0]:
                            pending.pop(0)[1]()
                        for fn in fillers.get(gcur[0], ()):
                            fn()
                        gcur[0] += 1
            pv_dn(prev[0])
            for _, fn in pending:
                fn()

    _split_waits(nc)
    return nc


def _get_program():
    global _PROGRAM
    if _PROGRAM is None:
        _PROGRAM = _build_program()
    return _PROGRAM


def _pack_chunks(wT, width):
    # [768, width] -> [128, 6*width] with chunk-major per-partition layout
    return np.ascontiguousarray(
        wT.reshape(NDC, 128, width).transpose(1, 0, 2).reshape(128, -1))


def kernel(x, Wq, Wk, Wv, Wo, bo):
    global LAST_RESULT
    from concourse.bass_utils import run_bass_kernel_spmd

    x = np.asarray(x, np.float32)
    Wq = np.asarray(Wq, np.float32)
    Wk = np.asarray(Wk, np.float32)
    Wv = np.asarray(Wv, np.float32)
    Wo = np.asarray(Wo, np.float32)
    bo = np.asarray(bo, np.float32)

    tri = np.tril(np.ones((128, 128), np.float32)).T  # 1 where j<=i
    mk = tri.astype(np.float16)

    in_maps = []
    for c in range(8):
        b, gi = divmod(c, 2)
        hs = slice(G * gi, G * (gi + 1))
        import ml_dtypes
        f8 = ml_dtypes.float8_e4m3

        xt = np.ascontiguousarray(x[b].T).astype(np.float16)
        xt3 = np.ascontiguousarray(
            xt.reshape(NDC, 128, NSC, W).transpose(1, 2, 0, 3))
        xt8 = xt3.astype(f8)
        wqT = Wq[hs, :].T.astype(np.float32)   # [768, 384]
        wkT = Wk[hs, :].T.astype(np.float32)
        wvT = Wv[hs, :].T.astype(np.float16)
        woT = Wo[:, hs].T.astype(np.float16)   # [384, 768]

        wqk8 = np.zeros((128, 2 * NPAIR, NDC, 128), f8)
        for pr in range(NPAIR):
            for t, wT in ((0, wqT), (1, wkT)):
                wqk8[:, 2 * pr + t] = (
                    wT[:, 128 * pr:128 * (pr + 1)] * SC8
                ).reshape(NDC, 128, 128).transpose(1, 0, 2).astype(f8)

        wts = np.concatenate([
            mk,
            _pack_chunks(wvT, G),
            np.ascontiguousarray(
                woT.reshape(3, 128, D).transpose(1, 0, 2).reshape(128, -1)),
        ], axis=1)
        assert wts.shape == (128, WX), wts.shape

        in_maps.append({
            "xt": np.ascontiguousarray(xt3),
            "xt8": xt8,
            "wqk8": np.ascontiguousarray(wqk8),
            "wts": np.ascontiguousarray(wts),
        })

    if PROFILE:
        _install_profile_hooks()
    nc = _get_program()
    res = run_bass_kernel_spmd(nc, in_maps, core_ids=list(range(8)),
                               trace=PROFILE, tmpdir=PROFILE_DIR)
    LAST_RESULT = res
    parts = [res.results[c]["y"] for c in range(8)]
    out = np.stack([parts[2 * b].astype(np.float32)
                    + parts[2 * b + 1].astype(np.float32)
                    + bo for b in range(B)])
    return out.astype(np.float32)
